# revision 13
# baseline (speedup 1.0000x reference)
"""DeepCausalQualityVGG loss kernel for 8 Trainium2 NeuronCores.

Structure:
  - Kernel A (device): VGG-16 forward for both images (13 convs + 4 hanning
    l2-pools, fp32), per-scale patch-pdf computation, patchified pdf tensors
    written to DRAM.
  - Kernel B (device, sharded across the 8 cores by column): per-(channel,
    patch) 1-D Wasserstein cost of the clean pdfs via a bitonic merge
    network + prefix-scan counting (exactly equal to sort+searchsorted on
    monotone cumsums), plus per-column L1 distance and sum statistics.
  - Host: deterministic intervention noise (threefry, CPU), exact
    replication of the noisy-step mask bits for the late scales, gating and
    the final scalar reduction.
"""

import os
import subprocess
import sys
import tempfile

import numpy as np

WIN = 8
STEP = 10
MAX_INTENSITY = 0.1
VGG_CFG = [(3, 64), (64, 64), (64, 128), (128, 128), (128, 256), (256, 256),
           (256, 256), (256, 512), (512, 512), (512, 512), (512, 512),
           (512, 512), (512, 512)]
STAGE_SPLITS = [2, 4, 7, 10, 13]

SCALES = [(3, 128), (64, 128), (128, 64), (256, 32), (512, 16), (512, 8)]
SCALE_M = [(h // WIN) * (h // WIN) for _, h in SCALES]
SCALE_COLS = [c * m for (c, _), m in zip(SCALES, SCALE_M)]
COL_OFF = np.concatenate([[0], np.cumsum(SCALE_COLS)]).astype(np.int64)
TOT_COLS = int(COL_OFF[-1])          # 32000
N_CORES = 8
COLS_PER_CORE = 4096
N_GROUPS = COLS_PER_CORE // 128      # 32
NOISY_SCALES = (2, 3, 4, 5)

_cache = {}


# --------------------------------------------------------------------------
# host-side helpers
# --------------------------------------------------------------------------

def _img27(img):
    """Stacked 9-tap x 3-channel shifted copies of the zero-padded image so a
    single matmul base offset serves all 27 K-rows."""
    C, H, W = img.shape
    Wp = W + 2
    p = np.zeros((C, H + 2, Wp), np.float32)
    p[:, 1:H + 1, 1:W + 1] = img
    flat = p.reshape(C, -1)
    L = flat.shape[1]
    m = 132
    ext = np.zeros((C, L + 2 * m), np.float32)
    ext[:, m:m + L] = flat
    rows = []
    for dy in (-1, 0, 1):
        for dx in (-1, 0, 1):
            off = dy * Wp + dx
            for ci in range(C):
                rows.append(ext[ci, m + off:m + off + L])
    return np.stack(rows)                     # (27, L)


def _weight_plan():
    plan = []
    col = 0
    for li, (cin, cout) in enumerate(VGG_CFG):
        entry = {"col0": col, "blocks": []}
        if li == 0:
            entry["blocks"].append((0, 0, 0, 27, 64, col))
            col += 64
        else:
            nci = (cin + 127) // 128
            nco = (cout + 127) // 128
            for co_c in range(nco):
                co_sz = min(128, cout - 128 * co_c)
                for ci_c in range(nci):
                    K = min(128, cin - 128 * ci_c)
                    for tap in range(9):
                        entry["blocks"].append((ci_c, tap, co_c, K, co_sz, col))
                        col += co_sz
        entry["ncols"] = col - entry["col0"]
        plan.append(entry)
    return plan, col


def _pack_weights(ws):
    plan, ncols = _weight_plan()
    wpk = np.zeros((128, ncols), np.float32)
    for li, (cin, cout) in enumerate(VGG_CFG):
        w = np.asarray(ws[li])
        for (ci_c, tap, co_c, K, co_sz, col) in plan[li]["blocks"]:
            dy, dx = tap // 3, tap % 3
            if li == 0:
                blk = np.zeros((27, 64), np.float32)
                for t in range(9):
                    tdy, tdx = t // 3, t % 3
                    blk[t * 3:(t + 1) * 3, :] = w[:, :, tdy, tdx].T
                wpk[:27, col:col + 64] = blk
            else:
                blk = w[128 * co_c:128 * co_c + co_sz,
                        128 * ci_c:128 * ci_c + K, dy, dx].T
                wpk[:K, col:col + co_sz] = blk
    return wpk, plan


def _pack_bias(bs):
    cols = []
    for li, (_, cout) in enumerate(VGG_CFG):
        for co_c in range((cout + 127) // 128):
            co_sz = min(128, cout - 128 * co_c)
            b = np.zeros(128, np.float32)
            b[:co_sz] = np.asarray(bs[li])[128 * co_c:128 * co_c + co_sz]
            cols.append(b)
    return np.stack(cols, axis=1)


def _gen_noise(tmpdir):
    script = r"""
import sys, numpy as np
import jax
jax.config.update("jax_platforms", "cpu")
import jax.numpy as jnp
out = sys.argv[1]
key = jax.random.key(42)
shapes = {2: (128, 64), 3: (256, 16), 4: (512, 4), 5: (512, 1)}
for k, (C, M) in shapes.items():
    kk = jax.random.fold_in(key, k)
    n = jax.random.normal(kk, (10, 1, C, M, 64), dtype=jnp.float32)
    np.save(f"{out}/noise{k}.npy", np.asarray(n))
"""
    env = dict(os.environ)
    env["JAX_PLATFORMS"] = "cpu"
    subprocess.run([sys.executable, "-c", script, tmpdir], check=True, env=env,
                   capture_output=True)
    return {k: np.load(os.path.join(tmpdir, f"noise{k}.npy"))
            for k in NOISY_SCALES}


def _searchsorted_scan(a_rows, q_rows):
    cols, n = a_rows.shape
    m = q_rows.shape[1]
    low = np.zeros((cols, m), np.int32)
    high = np.full((cols, m), n, np.int32)
    for _ in range(int(np.ceil(np.log2(n + 1)))):
        mid = (low + high) // 2
        amid = np.take_along_axis(a_rows, mid, axis=1)
        go_left = q_rows <= amid
        low = np.where(go_left, low, mid)
        high = np.where(go_left, mid, high)
    return high


def _wass_exact(u_w, v_w):
    cols, n = u_w.shape
    u_cum = np.cumsum(u_w, axis=1, dtype=np.float32)
    v_cum = np.cumsum(v_w, axis=1, dtype=np.float32)
    qs = np.sort(np.concatenate([u_cum, v_cum], axis=1), axis=1)
    uq = np.clip(_searchsorted_scan(u_cum, qs), 0, n - 1).astype(np.float32)
    vq = np.clip(_searchsorted_scan(v_cum, qs), 0, n - 1).astype(np.float32)
    delta = qs - np.concatenate([np.zeros((cols, 1), np.float32),
                                 qs[:, :-1]], axis=1)
    return (delta * (uq - vq) ** 2).sum(axis=1, dtype=np.float32)


# --------------------------------------------------------------------------
# kernel A: VGG + pdf patches
# --------------------------------------------------------------------------

def _build_kernel_a():
    import concourse.bacc as bacc
    import concourse.mybir as mybir
    from concourse.tile import TileContext

    plan, wcols = _weight_plan()
    nc = bacc.Bacc("TRN2", target_bir_lowering=False, debug=False,
                   num_devices=N_CORES)
    f32 = mybir.dt.float32
    AF = mybir.ActivationFunctionType
    A = mybir.AluOpType
    img27_d = nc.dram_tensor("img27", (2, 27, 130 * 130), f32,
                             kind="ExternalInput")
    imgs_d = nc.dram_tensor("imgs", (6, 128 * 128), f32, kind="ExternalInput")
    wpk_d = nc.dram_tensor("wpk", (128, wcols), f32, kind="ExternalInput")
    bpk_n = sum((c + 127) // 128 for _, c in VGG_CFG)
    bpk_d = nc.dram_tensor("bpk", (128, bpk_n), f32, kind="ExternalInput")
    pdfx_d = nc.dram_tensor("pdfx", (TOT_COLS, 64), f32, kind="ExternalOutput")
    pdfy_d = nc.dram_tensor("pdfy", (TOT_COLS, 64), f32, kind="ExternalOutput")
    f1_d = nc.dram_tensor("f1scr", (2, 64, 128 * 128), f32, kind="Internal")
    f2_d = nc.dram_tensor("f2scr", (2, 64, 128 * 128), f32, kind="Internal")

    HANG = np.hanning(5)[1:-1]
    G2 = np.outer(HANG, HANG)
    G2 = (G2 / G2.sum()).astype(np.float32)

    bias_col = {}
    ci = 0
    for li, (_, cout) in enumerate(VGG_CFG):
        for co_c in range((cout + 127) // 128):
            bias_col[(li, co_c)] = ci
            ci += 1

    # resident buffer tags for stages 2..5 (act buffers are zero-padded rings)
    RES_IN = {2: ["m1"], 3: ["m0"], 4: ["c2", "c3"], 5: ["c0", "c1"],
              6: ["c2", "c3"], 7: ["d3", "d4", "d5", "d6"],
              8: ["d1", "d2", "d7", "d8"], 9: ["d3", "d4", "d5", "d6"],
              10: ["e5", "e6", "e7", "e8"], 11: ["e1", "e2", "e3", "e4"],
              12: ["e5", "e6", "e7", "e8"]}
    # RES_IN[li] gives the OUTPUT tags of layer li; input tags = output of prev

    with TileContext(nc) as tc:
        with tc.tile_pool(name="main", bufs=1) as pool, \
             tc.tile_pool(name="stream", bufs=2) as spool, \
             tc.tile_pool(name="wts", bufs=2) as wpool, \
             tc.tile_pool(name="ps", bufs=4, space="PSUM") as psum:

            bias_t = pool.tile([128, bpk_n], f32, tag="bias")
            nc.sync.dma_start(bias_t[:], bpk_d[:])
            eps_t = pool.tile([128, 1], f32, tag="eps")
            nc.vector.memset(eps_t[:], 1e-12)

            def border_zero(t, cc, H, Wp):
                v = t[0:cc, 0:(H + 2) * Wp].rearrange(
                    "p (h w) -> p h w", h=H + 2, w=Wp)
                nc.vector.memset(v[:, 0:H + 2:H + 1, :], 0.0)
                nc.vector.memset(v[:, 1:H + 1, 0:Wp:Wp - 1], 0.0)

            def pdf_recip(cs_ap, nch):
                # cs_ap: [128, nch] per-channel-chunk partial sums
                tot = pool.tile([1, 8], f32, tag="tot")
                nc.gpsimd.tensor_reduce(tot[0:1, 0:nch], cs_ap,
                                        mybir.AxisListType.C, A.add)
                tot1 = pool.tile([1, 1], f32, tag="tot1")
                nc.vector.tensor_reduce(tot1[:], tot[0:1, 0:nch],
                                        mybir.AxisListType.X, A.add)
                s1 = pool.tile([1, 1], f32, tag="s1")
                nc.vector.tensor_scalar_add(s1[:], tot1[:], 1e-6)
                r0 = pool.tile([1, 1], f32, tag="r0")
                nc.vector.reciprocal(r0[:], s1[:])
                t1 = pool.tile([1, 1], f32, tag="t1")
                nc.vector.tensor_mul(t1[:], s1[:], r0[:])
                nc.vector.tensor_scalar(t1[:], t1[:], -1.0, 2.0,
                                        A.mult, A.add)
                nc.vector.tensor_mul(r0[:], r0[:], t1[:])
                rb = pool.tile([128, 1], f32, tag="rb")
                nc.gpsimd.partition_broadcast(rb[:], r0[0:1, 0:1])
                return rb

            def emit_pdf(src_view, cc, c_i, nb, mh, rb, scale_idx, out_dram):
                # src_view: [cc, WIN, W] feature rows of patch-row mh;
                # pdf computed in row layout, patch reorder done by the DMA
                pdf_s = spool.tile([128, 16 * 64], f32, tag="pdfs")
                ov = pdf_s[0:cc, 0:nb * 64].rearrange(
                    "p (h w) -> p h w", h=WIN, w=nb * WIN)
                nc.vector.scalar_tensor_tensor(ov, src_view, rb[0:cc, 0:1],
                                               src_view, A.mult, A.mult)
                W8 = nb * WIN * WIN          # 8*W elements per mh block
                dv = out_dram[int(COL_OFF[scale_idx]):
                              int(COL_OFF[scale_idx + 1]), :].rearrange(
                    "(c r) n -> c (r n)", c=min(999999, (int(COL_OFF[scale_idx + 1]) - int(COL_OFF[scale_idx])) // (nb * nb)))
                nc.sync.dma_start(
                    dv[128 * c_i:128 * c_i + cc, mh * W8:(mh + 1) * W8],
                    pdf_s[0:cc, 0:W8])

            # ---------- streaming stage-1 helpers (H = 128) ----------
            def conv_stream(li, img_idx, img27_t_unused=None):
                # li 0: img27 -> f1 ; li 1: f1 -> f2
                H = W = 128
                rpt = 4
                blocks = plan[li]["blocks"]
                wt = wpool.tile([128, plan[li]["ncols"]], f32, tag="w")
                nc.sync.dma_start(
                    wt[:], wpk_d[:, plan[li]["col0"]:
                                 plan[li]["col0"] + plan[li]["ncols"]])
                dst = f1_d if li == 0 else f2_d
                for tix in range(H // rpt):
                    h0 = tix * rpt
                    if li == 0:
                        st = spool.tile([27, rpt * 130], f32, tag="st0")
                        nc.sync.dma_start(
                            st[:], img27_d[img_idx, :,
                                           (h0 + 1) * 130:(h0 + 5) * 130])
                        rvb = st[0:27, :].rearrange("p (h w) -> p h w",
                                                    h=rpt, w=130)
                    else:
                        st = spool.tile([64, (rpt + 2) * 130], f32, tag="st1")
                        nc.vector.memset(st[:], 0.0)
                        lo = max(0, h0 - 1)
                        hi = min(H, h0 + rpt + 1)
                        sv = st[0:64, :].rearrange("p (h w) -> p h w",
                                                   h=rpt + 2, w=130)
                        nc.sync.dma_start(
                            sv[:, lo - (h0 - 1):hi - (h0 - 1), 1:129],
                            f1_d[img_idx, :, lo * 128:hi * 128].rearrange(
                                "p (h w) -> p h w", h=hi - lo, w=128))
                        rvb = sv
                    pt = psum.tile([128, rpt * 128], f32, tag="ps")
                    for bi, (ci_c, tap, _co, K, csz, col) in enumerate(blocks):
                        dy, dx = tap // 3 - 1, tap % 3 - 1
                        wv = wt[0:K, col - plan[li]["col0"]:
                                col - plan[li]["col0"] + csz]
                        if li == 0:
                            rv = rvb[:, :, 1:129]
                        else:
                            rv = rvb[:, 1 + dy:1 + dy + rpt, 1 + dx:1 + dx + 128]
                        nc.tensor.matmul(pt[0:64, :], wv, rv,
                                         start=(bi == 0),
                                         stop=(bi == len(blocks) - 1))
                    ot = spool.tile([64, rpt * 128], f32, tag="ot1")
                    nc.scalar.activation(ot[:], pt[0:64, :], AF.Relu,
                                         bias=bias_t[0:64, bias_col[(li, 0)]:
                                                     bias_col[(li, 0)] + 1],
                                         scale=1.0)
                    nc.sync.dma_start(dst[img_idx, :,
                                          h0 * 128:(h0 + rpt) * 128], ot[:])

            def pool1_stream(img_idx):
                # f2 (64ch,128x128) -> resident stage-2 buffer [64, 66*66] m0
                Ho = 64
                Wpo = 66
                ot = pool.tile([128, (Ho + 2) * Wpo], f32, tag="m0")
                border_zero(ot, 64, Ho, Wpo)
                for s0 in range(0, Ho, 8):
                    st = spool.tile([64, 18 * 130], f32, tag="pst")
                    nc.vector.memset(st[:], 0.0)
                    lo = max(0, 2 * s0 - 1)
                    hi = min(128, 2 * s0 + 16)
                    sv = st[0:64, :].rearrange("p (h w) -> p h w", h=18, w=130)
                    nc.sync.dma_start(
                        sv[:, lo - (2 * s0 - 1):hi - (2 * s0 - 1), 1:129],
                        f2_d[img_idx, :, lo * 128:hi * 128].rearrange(
                            "p (h w) -> p h w", h=hi - lo, w=128))
                    sq = spool.tile([64, 18 * 130], f32, tag="psq")
                    nc.scalar.activation(sq[:], st[:], AF.Square)
                    sqv = sq[0:64, :].rearrange("p (h w) -> p h w",
                                                h=18, w=130)
                    acc = spool.tile([64, 8 * 64], f32, tag="pac")
                    av = acc[0:64, :].rearrange("p (h w) -> p h w", h=8, w=64)
                    for k in range(9):
                        dy, dx = k // 3, k % 3
                        tv2 = sqv[:, dy:dy + 16:2, dx:dx + 128:2]
                        if k == 0:
                            nc.vector.tensor_scalar_mul(av, tv2,
                                                        float(G2[dy, dx]))
                        else:
                            nc.vector.scalar_tensor_tensor(
                                av, tv2, float(G2[dy, dx]), av,
                                A.mult, A.add)
                    ovv = ot[0:64, Wpo * (1 + s0) + 1:
                             Wpo * (1 + s0) + 1 + 8 * Wpo].rearrange(
                        "p (h w) -> p h w", h=8, w=Wpo)[:, :, 0:Ho]
                    nc.scalar.activation(ovv, av, AF.Sqrt,
                                         bias=eps_t[0:64, 0:1])
                return [ot]

            def tap_stream(img_idx, out_dram, scale_idx):
                # scale 0: from imgs (3 ch); scale 1: from f2 (64 ch)
                cc = 3 if scale_idx == 0 else 64
                def dma_block(st, mh):
                    if scale_idx == 0:
                        nc.sync.dma_start(
                            st[0:cc, :], imgs_d[3 * img_idx:3 * img_idx + 3,
                                                mh * 1024:(mh + 1) * 1024])
                    else:
                        nc.sync.dma_start(
                            st[0:cc, :],
                            f2_d[img_idx, :, mh * 1024:(mh + 1) * 1024])
                cs = pool.tile([128, 16], f32, tag="cs16")
                nc.vector.memset(cs[:], 0.0)
                for mh in range(16):
                    st = spool.tile([128, 8 * 128], f32, tag="stp")
                    dma_block(st, mh)
                    nc.vector.tensor_reduce(cs[0:cc, mh:mh + 1], st[0:cc, :],
                                            mybir.AxisListType.X, A.add)
                cs1 = pool.tile([128, 8], f32, tag="cs1")
                nc.vector.tensor_reduce(cs1[:, 0:1], cs[:],
                                        mybir.AxisListType.X, A.add)
                rb = pdf_recip(cs1[:, 0:1], 1)
                for mh in range(16):
                    st = spool.tile([128, 8 * 128], f32, tag="stp")
                    dma_block(st, mh)
                    sv = st[0:cc, :].rearrange("p (h w) -> p h w",
                                               h=8, w=128)
                    emit_pdf(sv, cc, 0, 16, mh, rb, scale_idx, out_dram)

            # ---------- resident helpers (stages 2..5) ----------
            def tap_pdf(feat_tiles, C, H, scale_idx, out_dram):
                W = H
                Wp = W + 2
                nch = len(feat_tiles)
                cs = pool.tile([128, 8], f32, tag="cs")
                nc.vector.memset(cs[:], 0.0)
                for c_i, ft in enumerate(feat_tiles):
                    cc = min(128, C - 128 * c_i)
                    iview = ft[0:cc, :].rearrange("p (h w) -> p h w",
                                                  h=H + 2, w=Wp)[
                        :, 1:H + 1, 1:W + 1]
                    nc.vector.tensor_reduce(cs[0:cc, c_i:c_i + 1], iview,
                                            mybir.AxisListType.XY, A.add)
                rb = pdf_recip(cs[:, 0:nch], nch)
                nb = H // WIN
                for c_i, ft in enumerate(feat_tiles):
                    cc = min(128, C - 128 * c_i)
                    fv = ft[0:cc, :].rearrange("p (h w) -> p h w",
                                               h=H + 2, w=Wp)
                    for mh in range(nb):
                        sv = fv[:, 1 + WIN * mh:1 + WIN * (mh + 1), 1:W + 1]
                        emit_pdf(sv, cc, c_i, nb, mh, rb, scale_idx, out_dram)

            def conv_layer(li, in_tiles, H):
                cin, cout = VGG_CFG[li]
                W = H
                Wp = W + 2
                nco = (cout + 127) // 128
                out_tiles = []
                for co_c in range(nco):
                    co_sz = min(128, cout - 128 * co_c)
                    ot = pool.tile([128, (H + 2) * Wp], f32,
                                   tag=RES_IN[li][co_c])
                    border_zero(ot, co_sz, H, Wp)
                    out_tiles.append(ot)
                rpt = max(1, 512 // W)
                ntile = (H + rpt - 1) // rpt
                for co_c in range(nco):
                    co_sz = min(128, cout - 128 * co_c)
                    bcol = bias_col[(li, co_c)]
                    blocks = [b for b in plan[li]["blocks"] if b[2] == co_c]
                    wci = 9 * ((cin + 127) // 128) * co_sz
                    wt = wpool.tile([128, wci], f32, tag="w")
                    nc.sync.dma_start(
                        wt[:], wpk_d[:, blocks[0][5]:blocks[0][5] + wci])
                    for tix in range(ntile):
                        h0 = tix * rpt
                        rr = min(rpt, H - h0)
                        pt = psum.tile([128, rpt * W], f32, tag="ps")
                        for bi, (ci_c, tap, _co, K, csz, col) in \
                                enumerate(blocks):
                            dy, dx = tap // 3 - 1, tap % 3 - 1
                            wv = wt[0:K, col - blocks[0][5]:
                                    col - blocks[0][5] + csz]
                            itv = in_tiles[ci_c][0:K, :].rearrange(
                                "p (h w) -> p h w", h=H + 2, w=Wp)
                            rv = itv[:, h0 + 1 + dy:h0 + 1 + dy + rr,
                                     1 + dx:1 + dx + W]
                            nc.tensor.matmul(pt[0:co_sz, 0:rr * W], wv, rv,
                                             start=(bi == 0),
                                             stop=(bi == len(blocks) - 1))
                        ovv = out_tiles[co_c][
                            0:co_sz, (h0 + 1) * Wp + 1:
                            (h0 + 1) * Wp + 1 + rr * Wp].rearrange(
                            "p (h w) -> p h w", h=rr, w=Wp)[:, :, 0:W]
                        nc.scalar.activation(
                            ovv,
                            pt[0:co_sz, 0:rr * W].rearrange(
                                "p (h w) -> p h w", h=rr, w=W),
                            AF.Relu,
                            bias=bias_t[0:co_sz, bcol:bcol + 1], scale=1.0)
                return out_tiles

            POOL_OUT = {4: ["c0"], 7: ["d1", "d2"],
                        10: ["e1", "e2", "e3", "e4"]}
            POOL_SQ = {4: "m1", 7: "c0", 10: "d9"}

            def l2pool(in_tiles, C, H, li_next):
                W = H
                Wp = W + 2
                Ho = H // 2
                Wpo = Ho + 2
                out_tiles = []
                nch_out = (C + 127) // 128
                for c_i, it in enumerate(in_tiles):
                    cc = min(128, C - 128 * c_i)
                    sq = pool.tile([128, (H + 2) * Wp], f32,
                                   tag=POOL_SQ[li_next])
                    border_zero(sq, cc, H, Wp)
                    iv = it[0:cc, :].rearrange("p (h w) -> p h w",
                                               h=H + 2, w=Wp)[
                        :, 1:H + 1, 1:W + 1]
                    sv = sq[0:cc, :].rearrange("p (h w) -> p h w",
                                               h=H + 2, w=Wp)[
                        :, 1:H + 1, 1:W + 1]
                    nc.scalar.activation(sv, iv, AF.Square)
                    acc = pool.tile([128, Ho * Ho], f32, tag="pacc")
                    sqv = sq[0:cc, :].rearrange("p (h w) -> p h w",
                                                h=H + 2, w=Wp)
                    av = acc[0:cc, :].rearrange("p (h w) -> p h w",
                                                h=Ho, w=Ho)
                    for k in range(9):
                        dy, dx = k // 3, k % 3
                        tv2 = sqv[:, dy:dy + 2 * Ho:2, dx:dx + 2 * Ho:2]
                        if k == 0:
                            nc.vector.tensor_scalar_mul(av, tv2,
                                                        float(G2[dy, dx]))
                        else:
                            nc.vector.scalar_tensor_tensor(
                                av, tv2, float(G2[dy, dx]), av,
                                A.mult, A.add)
                    ot = pool.tile([128, (Ho + 2) * Wpo], f32,
                                   tag=POOL_OUT[li_next][c_i])
                    border_zero(ot, cc, Ho, Wpo)
                    ovv = ot[0:cc, :].rearrange("p (h w) -> p h w",
                                                h=Ho + 2, w=Wpo)[
                        :, 1:Ho + 1, 1:Ho + 1]
                    nc.scalar.activation(ovv, av, AF.Sqrt,
                                         bias=eps_t[0:cc, 0:1])
                    out_tiles.append(ot)
                return out_tiles

            for img_idx in range(2):
                out_dram = pdfx_d if img_idx == 0 else pdfy_d
                tap_stream(img_idx, out_dram, 0)
                conv_stream(0, img_idx)
                conv_stream(1, img_idx)
                tap_stream(img_idx, out_dram, 1)
                tiles = pool1_stream(img_idx)
                H = 64
                li = 2
                for stage, end in list(enumerate(STAGE_SPLITS))[1:]:
                    if stage > 1:
                        C = VGG_CFG[li - 1][1]
                        tiles = l2pool(tiles, C, H, li)
                        H //= 2
                    while li < end:
                        tiles = conv_layer(li, tiles, H)
                        li += 1
                    C = VGG_CFG[li - 1][1]
                    tap_pdf(tiles, C, H, stage + 1, out_dram)
    nc.compile()
    return nc


# --------------------------------------------------------------------------
# kernel B: clean wasserstein columns
# --------------------------------------------------------------------------

def _build_kernel_b():
    import concourse.bacc as bacc
    import concourse.mybir as mybir
    from concourse.tile import TileContext

    nc = bacc.Bacc("TRN2", target_bir_lowering=False, debug=False,
                   num_devices=N_CORES)
    f32 = mybir.dt.float32
    u_d = nc.dram_tensor("u", (N_GROUPS, 128, 64), f32, kind="ExternalInput")
    v_d = nc.dram_tensor("v", (N_GROUPS, 128, 64), f32, kind="ExternalInput")
    cost_d = nc.dram_tensor("cost", (128, N_GROUPS), f32,
                            kind="ExternalOutput")
    md_d = nc.dram_tensor("md", (128, N_GROUPS), f32, kind="ExternalOutput")
    sd_d = nc.dram_tensor("sd", (128, N_GROUPS), f32, kind="ExternalOutput")
    ss_d = nc.dram_tensor("ss", (128, N_GROUPS), f32, kind="ExternalOutput")

    A = mybir.AluOpType
    X = mybir.AxisListType.X
    with TileContext(nc) as tc:
        with tc.tile_pool(name="p", bufs=2) as pool, \
             tc.tile_pool(name="acc", bufs=1) as accp:
            cost_a = accp.tile([128, N_GROUPS], f32, tag="cost")
            md_a = accp.tile([128, N_GROUPS], f32, tag="md")
            sd_a = accp.tile([128, N_GROUPS], f32, tag="sd")
            ss_a = accp.tile([128, N_GROUPS], f32, tag="ss")
            iota_i = accp.tile([128, 128], mybir.dt.int32, tag="iotai")
            nc.gpsimd.iota(iota_i[:], [[1, 128]], channel_multiplier=0)
            iota = accp.tile([128, 128], f32, tag="iota")
            nc.vector.tensor_copy(iota[:], iota_i[:])

            for g in range(N_GROUPS):
                ut = pool.tile([128, 64], f32, tag="ut")
                vt = pool.tile([128, 64], f32, tag="vt")
                nc.sync.dma_start(ut[:], u_d[g])
                nc.sync.dma_start(vt[:], v_d[g])
                sdt = pool.tile([128, 64], f32, tag="sdt")
                nc.vector.tensor_sub(sdt[:], ut[:], vt[:])
                nc.vector.tensor_reduce(sd_a[:, g:g + 1], sdt[:], X, A.add)
                nc.vector.tensor_reduce(md_a[:, g:g + 1], sdt[:], X, A.add,
                                        apply_absolute_value=True)
                sst = pool.tile([128, 64], f32, tag="sst")
                nc.vector.tensor_add(sst[:], ut[:], vt[:])
                nc.vector.tensor_reduce(ss_a[:, g:g + 1], sst[:], X, A.add)
                zz = pool.tile([128, 64], f32, tag="zz")
                nc.vector.memset(zz[:], 0.0)
                uc = pool.tile([128, 64], f32, tag="uc")
                vc = pool.tile([128, 64], f32, tag="vc")
                nc.vector.tensor_tensor_scan(uc[:], ut[:], zz[:], 0.0,
                                             A.add, A.add)
                nc.vector.tensor_tensor_scan(vc[:], vt[:], zz[:], 0.0,
                                             A.add, A.add)
                ka = pool.tile([128, 128], f32, tag="ka")
                pa = pool.tile([128, 128], f32, tag="pa")
                vrev = vc[:, ::-1]
                m1 = pool.tile([128, 64], f32, tag="m1")
                nc.vector.tensor_tensor(m1[:], uc[:], vrev, A.is_le)
                nc.vector.tensor_tensor(ka[:, 0:64], uc[:], vrev, A.min)
                nc.vector.tensor_tensor(ka[:, 64:128], uc[:], vrev, A.max)
                nc.vector.tensor_copy(pa[:, 0:64], m1[:])
                nc.vector.tensor_scalar(pa[:, 64:128], m1[:], -1.0, 1.0,
                                        A.mult, A.add)
                kb = pool.tile([128, 128], f32, tag="kb")
                pb = pool.tile([128, 128], f32, tag="pb")
                cur_k, cur_p, nxt_k, nxt_p = ka, pa, kb, pb
                s = 32
                while s >= 1:
                    nblk = 64 // s
                    kv = cur_k[:].rearrange("p (b t x) -> p b t x",
                                            b=nblk, t=2, x=s)
                    pv = cur_p[:].rearrange("p (b t x) -> p b t x",
                                            b=nblk, t=2, x=s)
                    ko = nxt_k[:].rearrange("p (b t x) -> p b t x",
                                            b=nblk, t=2, x=s)
                    po = nxt_p[:].rearrange("p (b t x) -> p b t x",
                                            b=nblk, t=2, x=s)
                    av, bv = kv[:, :, 0, :], kv[:, :, 1, :]
                    pav, pbv = pv[:, :, 0, :], pv[:, :, 1, :]
                    ms = pool.tile([128, 128], f32, tag="ms")
                    msv = ms[:].rearrange("p (b t x) -> p b t x",
                                          b=nblk, t=2, x=s)
                    mv, sc0 = msv[:, :, 0, :], msv[:, :, 1, :]
                    nc.vector.tensor_tensor(mv, av, bv, A.is_le)
                    nc.vector.tensor_tensor(ko[:, :, 0, :], av, bv, A.min)
                    nc.vector.tensor_tensor(ko[:, :, 1, :], av, bv, A.max)
                    # payload select via arithmetic (payloads are exactly 0/1)
                    nc.vector.tensor_tensor(sc0, pav, pbv, A.subtract)
                    nc.vector.tensor_tensor(sc0, mv, sc0, A.mult)
                    nc.vector.tensor_tensor(po[:, :, 0, :], pbv, sc0, A.add)
                    nc.vector.tensor_tensor(sc0, pav, pbv, A.add)
                    nc.vector.tensor_tensor(po[:, :, 1, :], sc0,
                                            po[:, :, 0, :], A.subtract)
                    cur_k, nxt_k = nxt_k, cur_k
                    cur_p, nxt_p = nxt_p, cur_p
                    s //= 2
                qs, I = cur_k, cur_p
                r = pool.tile([128, 128], f32, tag="r")
                zz2 = pool.tile([128, 128], f32, tag="zz2")
                nc.vector.memset(zz2[:], 0.0)
                nc.vector.tensor_tensor_scan(r[:], I[:], zz2[:], 0.0,
                                             A.add, A.add)
                ne = pool.tile([128, 128], f32, tag="ne")
                nc.vector.memset(ne[:, 0:1], 1.0)
                nc.vector.tensor_tensor(ne[:, 1:128], qs[:, 1:128],
                                        qs[:, 0:127], A.not_equal)
                lat = pool.tile([128, 128], f32, tag="lat")
                nc.vector.memset(lat[:, 0:1], 0.0)
                t1 = pool.tile([128, 128], f32, tag="t1b")
                nc.vector.tensor_scalar_add(t1[:, 1:128], r[:, 0:127], 1.0)
                nc.vector.tensor_mul(lat[:, 1:128], ne[:, 1:128],
                                     t1[:, 1:128])
                nc.vector.tensor_scalar_add(lat[:, 1:128], lat[:, 1:128],
                                            -1.0)
                uqt = pool.tile([128, 128], f32, tag="uqt")
                nc.vector.tensor_tensor_scan(uqt[:], lat[:], zz2[:], 0.0,
                                             A.max, A.add)
                nc.vector.tensor_scalar_min(uqt[:], uqt[:], 63.0)
                latv = pool.tile([128, 128], f32, tag="latv")
                nc.vector.memset(latv[:, 0:1], 0.0)
                t2 = pool.tile([128, 128], f32, tag="t2b")
                nc.vector.tensor_sub(t2[:, 1:128], iota[:, 1:128],
                                     r[:, 0:127])
                nc.vector.tensor_scalar_add(t2[:, 1:128], t2[:, 1:128], 1.0)
                nc.vector.tensor_mul(latv[:, 1:128], ne[:, 1:128],
                                     t2[:, 1:128])
                nc.vector.tensor_scalar_add(latv[:, 1:128], latv[:, 1:128],
                                            -1.0)
                vqt = pool.tile([128, 128], f32, tag="vqt")
                nc.vector.tensor_tensor_scan(vqt[:], latv[:], zz2[:], 0.0,
                                             A.max, A.add)
                nc.vector.tensor_scalar_min(vqt[:], vqt[:], 63.0)
                d = pool.tile([128, 128], f32, tag="d")
                nc.vector.tensor_sub(d[:], uqt[:], vqt[:])
                f = pool.tile([128, 128], f32, tag="f")
                nc.vector.tensor_mul(f[:], d[:], d[:])
                dl = pool.tile([128, 128], f32, tag="dl")
                nc.vector.tensor_copy(dl[:, 0:1], qs[:, 0:1])
                nc.vector.tensor_sub(dl[:, 1:128], qs[:, 1:128],
                                     qs[:, 0:127])
                pr = pool.tile([128, 128], f32, tag="pr")
                nc.vector.tensor_mul(pr[:], dl[:], f[:])
                nc.vector.tensor_reduce(cost_a[:, g:g + 1], pr[:], X, A.add)
            nc.sync.dma_start(cost_d[:], cost_a[:])
            nc.sync.dma_start(md_d[:], md_a[:])
            nc.sync.dma_start(sd_d[:], sd_a[:])
            nc.sync.dma_start(ss_d[:], ss_a[:])
    nc.compile()
    return nc


# --------------------------------------------------------------------------
# main entry
# --------------------------------------------------------------------------

def kernel(x, y, ws, bs, _profile=None):
    from concourse import bass_utils

    x = np.asarray(x, np.float32)
    y = np.asarray(y, np.float32)

    if "a" not in _cache:
        _cache["a"] = _build_kernel_a()
    if "b" not in _cache:
        _cache["b"] = _build_kernel_b()

    img27 = np.stack([_img27(x[0]), _img27(y[0])])
    imgs = np.concatenate([x[0].reshape(3, -1), y[0].reshape(3, -1)])
    wpk, _ = _pack_weights(ws)
    bpk = _pack_bias(bs)

    import time as _time

    def _run(nc, in_maps, want_trace):
        if want_trace:
            try:
                return bass_utils.run_bass_kernel_spmd(
                    nc, in_maps, core_ids=list(range(N_CORES)), trace=True)
            except Exception:
                pass
        return bass_utils.run_bass_kernel_spmd(
            nc, in_maps, core_ids=list(range(N_CORES)))

    in_a = dict(img27=img27, imgs=imgs, wpk=wpk, bpk=bpk)
    _t0 = _time.perf_counter()
    res_a = _run(_cache["a"], [in_a] * N_CORES, bool(_profile))
    def to_patches(flat):
        out = np.empty((TOT_COLS, 64), np.float32)
        for s, ((C, H), M) in enumerate(zip(SCALES, SCALE_M)):
            sl = slice(int(COL_OFF[s]), int(COL_OFF[s + 1]))
            nb = H // WIN
            f = flat[sl].reshape(C, H, H)
            out[sl] = (f.reshape(C, nb, WIN, nb, WIN)
                       .transpose(0, 1, 3, 2, 4).reshape(C * M, 64))
        return out

    pdfx = to_patches(res_a.results[0]["pdfx"])
    pdfy = to_patches(res_a.results[0]["pdfy"])
    if _profile is not None:
        _profile["a_ns"] = res_a.exec_time_ns
        _profile["a_wall_ns"] = int((_time.perf_counter() - _t0) * 1e9)

    up = np.zeros((N_CORES * COLS_PER_CORE, 64), np.float32)
    vp = np.zeros_like(up)
    up[:TOT_COLS] = pdfx
    vp[:TOT_COLS] = pdfy
    in_bs = []
    for c in range(N_CORES):
        sl = slice(c * COLS_PER_CORE, (c + 1) * COLS_PER_CORE)
        in_bs.append(dict(u=up[sl].reshape(N_GROUPS, 128, 64).copy(),
                          v=vp[sl].reshape(N_GROUPS, 128, 64).copy()))
    _t1 = _time.perf_counter()
    res_b = _run(_cache["b"], in_bs, bool(_profile))
    if _profile is not None:
        _profile["b_ns"] = res_b.exec_time_ns
        _profile["b_wall_ns"] = int((_time.perf_counter() - _t1) * 1e9)

    def unpack(name):
        out = np.empty(N_CORES * COLS_PER_CORE, np.float32)
        for c in range(N_CORES):
            arr = res_b.results[c][name]
            out[c * COLS_PER_CORE:(c + 1) * COLS_PER_CORE] = arr.T.reshape(-1)
        return out[:TOT_COLS]

    cost = unpack("cost")
    mdc = unpack("md")
    sdc = unpack("sd")
    ssc = unpack("ss")

    with tempfile.TemporaryDirectory() as td:
        noise = _gen_noise(td)

    intensity = (np.arange(STEP, dtype=np.float32)
                 * np.float32(MAX_INTENSITY / STEP))
    total = np.float32(0.0)
    for s, ((C, H), M) in enumerate(zip(SCALES, SCALE_M)):
        sl = slice(int(COL_OFF[s]), int(COL_OFF[s + 1]))
        ot_c = cost[sl].reshape(C, M).sum(axis=1, dtype=np.float32)
        md_ch = mdc[sl].reshape(C, M).sum(axis=1, dtype=np.float32)
        thresh = np.float32(abs(sdc[sl].sum(dtype=np.float32) * 0.5) * 1e-4)
        fire = np.zeros(C, bool)
        if s in NOISY_SCALES:
            mean_xy = np.float32(ssc[sl].sum(dtype=np.float32) * 0.5
                                 / (C * M * 64))
            u = pdfx[sl].reshape(C * M, 64)
            v = pdfy[sl].reshape(C * M, 64)
            nz = noise[s].reshape(STEP, C * M, 64)
            for k in range(1, STEP):
                dist = (nz[k] * mean_xy * intensity[k]).astype(np.float32)
                xd = (u + dist).astype(np.float32)
                yd = (v + dist).astype(np.float32)
                otd = _wass_exact(xd, yd).reshape(C, M).sum(
                    axis=1, dtype=np.float32)
                fire |= np.abs(ot_c - otd) < thresh
        gate = ~((md_ch == 0) | fire)
        total += np.float32((ot_c[gate] + md_ch[gate]).sum(dtype=np.float32))
    return np.array([np.float32(total / 6.0)], np.float32)
